# revision 20
# baseline (speedup 1.0000x reference)
"""Multi-head attention (sparse_attention nn_Attention) on 8 TRN2 NeuronCores.

Reference computes standard MHA: project q/k/v, S = qh @ kh^T, attn =
softmax(S * scale), x = attn @ vh; returns (x, attn). The relation_feature
branch in the reference is dead code (computed then deleted), so
relation_feature is never touched here.

Sharding: 8 cores = 4 batches x 2 head-groups. Core c handles batch c//2,
heads [4*(c%2), 4*(c%2)+4). Weight slices per head-group, activations per
batch. No collectives: every core writes a disjoint output slice.

Host pre-transposes inputs so every matmul contraction lands on SBUF
partitions, packed [128, KC, 256] partition-major so each DMA partition line
is one contiguous chunk:
  qT/kT/vT  <- q[b].T              (C=512 rows -> KC=4 chunks of 128)
  w{q,k,v}T <- W[hg*256:+256, :].T
Per-core compute (querytok on partitions through softmax):
  qhT/khT [chan, tok], vh [tok, chan] projections on PE
  S[h]    [qtok, ktok] = qhT_h.T @ khT_h      (K=64 per head)
  U       = exp(S*scale) on ACT with fused row-sum (accum_out)
  attn    = U * (1/rowsum)  -> packed [qtok, h, ktok] f32 out
  UT      = PE-transpose(U);  xu = UT.T @ vh;  x = xu * (1/rowsum)
Matmul dtype modes: "float32" (4 cyc/row), "bfloat16" (1 cyc/row, bf16 DMA),
"float32r" (f32 storage bitcast to the fast PE path, 1 cyc/row at N>=256).
"""

import numpy as np

B, N, C = 4, 256, 512
H, HS = 8, 64
HG = 2  # head groups (tensor-parallel over heads)
GH = H // HG  # heads per group = 4
GC = GH * HS  # channels per group = 256
SCALE = HS**-0.5
P = 128
KC = C // P  # 4 contraction chunks
TQ = N // P  # 2 query-token chunks
TK = N // P  # 2 key-token chunks
CCH = GC // P  # 2 channel chunks per group

DT_MM = "bfloat16"  # "float32" | "bfloat16" | "float32r"
MODE = "raw"  # "raw" (manual semaphores) | "tile" (TileContext)

_CACHE = {}


def _np_in_dtype(dt_mm_name):
    if dt_mm_name == "bfloat16":
        import ml_dtypes

        return ml_dtypes.bfloat16
    return np.float32


def _make_slim_tile(tile):
    """TileContext whose exit skips the two all-engine barriers and ~57
    semaphore clears (~8us tail). Keeps the sync-engine drain (with waits on
    the global clock) so output DMAs complete before the NEFF ends. The NEFF
    then assumes clean semaphores at load (single execution per load, which
    is how run_bass_via_pjrt executes it)."""
    from concourse.vector_clock import ScopedClock

    class SlimTile(tile.TileContext):
        def _drain_and_barrier(self, tick_clock, wait_clock):
            drain_inst = self.nc.sync.drain()
            wait_clock.add_sem_waits(
                drain_inst.ins, ScopedClock({None: tick_clock.global_clock})
            )
            popped = self.nc._tile_sem_poison_stack.pop()
            assert popped is self._sem_poison

    return SlimTile


def _build_nc(dt_mm_name: str):
    import concourse.bass as bass  # noqa: F401
    import concourse.mybir as mybir
    import concourse.tile as tile
    from concourse import bacc
    from concourse.masks import make_identity

    f32 = mybir.dt.float32
    dt_mm = getattr(mybir.dt, dt_mm_name)
    # dtype of DRAM inputs + SBUF input tiles; f32r is stored as f32 in
    # SBUF/PSUM and bitcast to float32r at each matmul AP
    dt_in = mybir.dt.bfloat16 if dt_mm_name == "bfloat16" else f32
    dt_store = mybir.dt.bfloat16 if dt_mm_name == "bfloat16" else f32

    def mm_cast(ap):
        return ap.bitcast(dt_mm) if dt_mm_name == "float32r" else ap

    nc = bacc.Bacc("TRN2", target_bir_lowering=False)

    qT = nc.dram_tensor("qT", [P, KC, N], dt_in, kind="ExternalInput")
    kT = nc.dram_tensor("kT", [P, KC, N], dt_in, kind="ExternalInput")
    vT = nc.dram_tensor("vT", [P, KC, N], dt_in, kind="ExternalInput")
    wqT = nc.dram_tensor("wqT", [P, KC, GC], dt_in, kind="ExternalInput")
    wkT = nc.dram_tensor("wkT", [P, KC, GC], dt_in, kind="ExternalInput")
    wvT = nc.dram_tensor("wvT", [P, KC, GC], dt_in, kind="ExternalInput")
    # attn packed [querytok, head, keytok] so DMA rows are 4KB; host unpacks
    out_attn = nc.dram_tensor("out_attn", [N, GH, N], f32, kind="ExternalOutput")
    out_x = nc.dram_tensor("out_x", [N, GC], f32, kind="ExternalOutput")

    SlimTile = _make_slim_tile(tile)
    with SlimTile(nc) as tc:
        with (
            tc.tile_pool(name="inputs", bufs=1) as inp,
            tc.tile_pool(name="proj", bufs=1) as proj,
            tc.tile_pool(name="work", bufs=2) as work,
            tc.tile_pool(name="small", bufs=16) as small,
            tc.tile_pool(name="psA", bufs=2, space="PSUM") as psA,
            tc.tile_pool(name="psB", bufs=2, space="PSUM") as psB,
            tc.tile_pool(name="psC", bufs=2, space="PSUM") as psC,
        ):
            ident = inp.tile([P, P], dt_store)
            make_identity(nc, ident)

            def load(t, fdim):
                sb = inp.tile([P, KC, fdim], dt_in, tag=f"in_{t.name}")
                nc.sync.dma_start(sb[:], t[:])
                return sb

            wqT_sb = load(wqT, GC)
            qT_sb = load(qT, N)
            wkT_sb = load(wkT, GC)
            kT_sb = load(kT, N)
            wvT_sb = load(wvT, GC)
            vT_sb = load(vT, N)

            # ---- projections
            qhT = proj.tile([P, CCH, N], dt_store)  # [chan_part, cc, querytok]
            khT = proj.tile([P, CCH, N], dt_store)
            vh = proj.tile([P, TK, GC], dt_store)  # [keytok_part, tk, chan]

            def project(dst_slice, w_sb, x_sb, m_sl, nfree):
                ps_full = psA.tile([P, 512], f32, tag="proj_ps", name="proj_ps")
                ps = ps_full[:, :nfree]
                for kc in range(KC):
                    nc.tensor.matmul(
                        ps,
                        lhsT=mm_cast(w_sb[:, kc, m_sl]),
                        rhs=mm_cast(x_sb[:, kc, :]),
                        start=(kc == 0),
                        stop=(kc == KC - 1),
                    )
                nc.any.tensor_copy(dst_slice, ps)

            for cc in range(CCH):
                sl = slice(cc * P, (cc + 1) * P)
                project(qhT[:, cc, :], wqT_sb, qT_sb, sl, N)
                project(khT[:, cc, :], wkT_sb, kT_sb, sl, N)
            for tk in range(TK):
                sl = slice(tk * P, (tk + 1) * P)
                project(vh[:, tk, :], vT_sb, wvT_sb, sl, GC)

            # ---- attention: S, exp(+rowsum), normalize, write attn
            u_all = proj.tile([P, TQ, GH, N], dt_store)  # exp(S*scale)
            ut_all = proj.tile([P, GH, TK, TQ * P], dt_store)  # U^T
            recips = {}
            for tq in range(TQ):
                attn_f = work.tile([P, GH, N], f32, tag="attn_f")
                for h in range(GH):
                    cc = h // (P // HS)
                    r0 = (h % (P // HS)) * HS
                    s_ps = psA.tile([P, N], f32, tag="s_ps")
                    nc.tensor.matmul(
                        s_ps,
                        lhsT=mm_cast(qhT[r0 : r0 + HS, cc, tq * P : (tq + 1) * P]),
                        rhs=mm_cast(khT[r0 : r0 + HS, cc, :]),
                        start=True,
                        stop=True,
                    )
                    u = u_all[:, tq, h, :]
                    dsum = small.tile([P, 1], f32, tag="dsum")
                    nc.scalar.activation(
                        u,
                        s_ps,
                        mybir.ActivationFunctionType.Exp,
                        scale=SCALE,
                        accum_out=dsum,
                    )
                    rc = small.tile([P, 1], f32, tag="recip", name=f"rc_{tq}_{h}")
                    nc.vector.reciprocal(rc, dsum)
                    recips[tq, h] = rc
                    nc.vector.tensor_scalar_mul(attn_f[:, h, :], u, rc)
                nc.sync.dma_start(out_attn[tq * P : (tq + 1) * P, :, :], attn_f[:])

            # ---- transpose unnormalized U: [qtok, ktok] -> [ktok, qtok]
            for h in range(GH):
                for tk in range(TK):
                    t_ps = psB.tile([P, TQ * P], dt_store, tag="t_ps")
                    for tq in range(TQ):
                        nc.tensor.transpose(
                            mm_cast(t_ps[:, tq * P : (tq + 1) * P]),
                            mm_cast(u_all[:, tq, h, tk * P : (tk + 1) * P]),
                            mm_cast(ident[:, :]),
                        )
                    nc.any.tensor_copy(ut_all[:, h, tk, :], t_ps)

            # ---- x = attn @ vh (unnormalized, then scaled by recip)
            for tq in range(TQ):
                x_ps = psC.tile([P, GC], f32, tag="x_ps")
                for h in range(GH):
                    for tk in range(TK):
                        nc.tensor.matmul(
                            x_ps[:, h * HS : (h + 1) * HS],
                            lhsT=mm_cast(ut_all[:, h, tk, tq * P : (tq + 1) * P]),
                            rhs=mm_cast(vh[:, tk, h * HS : (h + 1) * HS]),
                            start=(tk == 0),
                            stop=(tk == TK - 1),
                        )
                x_sb = work.tile([P, GC], f32, tag="x_sb")
                for h in range(GH):
                    nc.vector.tensor_scalar_mul(
                        x_sb[:, h * HS : (h + 1) * HS],
                        x_ps[:, h * HS : (h + 1) * HS],
                        recips[tq, h],
                    )
                nc.sync.dma_start(out_x[tq * P : (tq + 1) * P, :], x_sb[:])

    nc.compile()
    return nc


def _build_nc_raw(dt_mm_name: str):
    """Raw bacc build: one Block, five engine streams, manual semaphores.

    Avoids TileContext's per-instruction semaphore plumbing and its
    barrier-heavy exit. PE is pre-warmed with dummy matmuls (HAM clock ramps
    to 2.4 GHz during the input DMA wait). Cross-engine sync via five
    counting semaphores: DS (dma in), OS (dma out), PS (tensor), AS (scalar),
    VS (vector). Program order within each engine carries the rest.
    """
    assert dt_mm_name == "bfloat16", "raw mode is bf16-only"
    import concourse.mybir as mybir
    from concourse import bacc

    f32 = mybir.dt.float32
    bf16 = mybir.dt.bfloat16
    Exp = mybir.ActivationFunctionType.Exp

    nc = bacc.Bacc("TRN2", target_bir_lowering=False)

    qT = nc.dram_tensor("qT", [P, KC, N], bf16, kind="ExternalInput")
    kT = nc.dram_tensor("kT", [P, KC, N], bf16, kind="ExternalInput")
    vT = nc.dram_tensor("vT", [P, KC, N], bf16, kind="ExternalInput")
    wqT = nc.dram_tensor("wqT", [P, KC, GC], bf16, kind="ExternalInput")
    wkT = nc.dram_tensor("wkT", [P, KC, GC], bf16, kind="ExternalInput")
    wvT = nc.dram_tensor("wvT", [P, KC, GC], bf16, kind="ExternalInput")
    identD = nc.dram_tensor("identD", [P, P], bf16, kind="ExternalInput")
    out_attn = nc.dram_tensor("out_attn", [N, GH, N], f32, kind="ExternalOutput")
    out_x = nc.dram_tensor("out_x", [N, GC], f32, kind="ExternalOutput")

    # SBUF
    ident_sb = nc.alloc_sbuf_tensor("ident_sb", [P, P], bf16)
    ins_sb = {
        "wqT": nc.alloc_sbuf_tensor("wq_sb", [P, KC, GC], bf16),
        "qT": nc.alloc_sbuf_tensor("q_sb", [P, KC, N], bf16),
        "wkT": nc.alloc_sbuf_tensor("wk_sb", [P, KC, GC], bf16),
        "kT": nc.alloc_sbuf_tensor("k_sb", [P, KC, N], bf16),
        "wvT": nc.alloc_sbuf_tensor("wv_sb", [P, KC, GC], bf16),
        "vT": nc.alloc_sbuf_tensor("v_sb", [P, KC, N], bf16),
    }
    qhT = nc.alloc_sbuf_tensor("qhT", [P, CCH, N], bf16)
    khT = nc.alloc_sbuf_tensor("khT", [P, CCH, N], bf16)
    vh = nc.alloc_sbuf_tensor("vh", [P, TK, GC], bf16)
    u_sb = nc.alloc_sbuf_tensor("u_sb", [P, TQ, GH, N], bf16)
    ut_sb = nc.alloc_sbuf_tensor("ut_sb", [P, GH, TK, TQ * P], bf16)
    attn_sb = nc.alloc_sbuf_tensor("attn_sb", [P, TQ, GH, N], f32)
    x_sb = nc.alloc_sbuf_tensor("x_sb", [P, TQ, GC], f32)
    ds_sb = nc.alloc_sbuf_tensor("ds_sb", [P, TQ * GH], f32)
    scratch = nc.alloc_sbuf_tensor("scratch", [P, 1], f32)
    rc_sb = nc.alloc_sbuf_tensor("rc_sb", [P, TQ * GH], f32)

    # PSUM: exactly 8 banks
    pq = [nc.alloc_psum_tensor(f"pq{i}", [P, 512], f32) for i in range(2)]
    ps = [nc.alloc_psum_tensor(f"ps{i}", [P, N], f32) for i in range(2)]
    pt = [nc.alloc_psum_tensor(f"pt{i}", [P, TQ * P], bf16) for i in range(2)]
    px = [nc.alloc_psum_tensor(f"px{i}", [P, GC], f32) for i in range(2)]

    DI = nc.alloc_semaphore("DI")  # ident DMA
    DQ = nc.alloc_semaphore("DQ")  # wq+q DMAs
    DK = nc.alloc_semaphore("DK")  # wk+k DMAs
    DV = nc.alloc_semaphore("DV")  # wv+v DMAs
    OS = nc.alloc_semaphore("OS")
    PS = nc.alloc_semaphore("PS")
    AS = nc.alloc_semaphore("AS")
    VS = nc.alloc_semaphore("VS")

    # --- static schedule bookkeeping (python-side counters) ---
    # (sem, target-count) per input DMA; waits use group totals so the
    # out-of-order completion of DMAs within a group cannot race the wait
    dma_in_order = ["identD", "wqT", "qT", "wkT", "kT", "wvT", "vT"]
    dma_sem = {"identD": DI, "wqT": DQ, "qT": DQ, "wkT": DK, "kT": DK,
               "wvT": DV, "vT": DV}
    dma_gate = {"identD": (DI, 16), "q": (DQ, 32), "k": (DK, 32), "v": (DV, 32)}
    dram_in = {
        "identD": identD, "wqT": wqT, "qT": qT, "wkT": wkT, "kT": kT,
        "wvT": wvT, "vT": vT,
    }

    # PE program order; PS semaphore value = 1 + index in this list
    pe_order = []
    for which in ("q", "k", "v"):
        for j in range(2):
            pe_order.append(("proj", which, j))
    for tq in range(TQ):
        for h in range(GH):
            pe_order.append(("S", tq, h))
    for h in range(GH):
        pe_order.append(("T", h, 0))
        pe_order.append(("T", h, 1))
        if h > 0:
            pe_order.append(("AV", 0, h - 1))
    pe_order.append(("AV", 0, GH - 1))
    for h in range(GH):
        pe_order.append(("AV", 1, h))
    ps_at = {op: i + 1 for i, op in enumerate(pe_order)}
    ps_proj = {(w, j): ps_at["proj", w, j] for w in ("q", "k", "v") for j in range(2)}
    ps_S = {(tq, h): ps_at["S", tq, h] for tq in range(TQ) for h in range(GH)}
    ps_T = {(h, tk): ps_at["T", h, tk] for h in range(GH) for tk in range(TK)}
    ps_AV = {(tq, h): ps_at["AV", tq, h] for tq in range(TQ) for h in range(GH)}

    as_exp = {}  # (tq,h) -> AS value
    n = 0
    for tq in range(TQ):
        for h in range(GH):
            n += 1
            as_exp[tq, h] = n

    # DVE order: 6 proj copies; tq0 recip+mul x4; tq1 (recip+mul
    # interleaved with ut copies); x muls. VS counters assigned in order.
    vs = {"n": 0}
    vs_at = {}

    def vs_next(key):
        vs["n"] += 1
        vs_at[key] = vs["n"]
        return vs["n"]

    # pre-assign the full DVE program order so producers know their targets
    dve_order = []
    for which, j in (("q", 0), ("q", 1), ("k", 0), ("k", 1), ("v", 0), ("v", 1)):
        dve_order.append(("copy_proj", which, j))
    for h in range(GH):
        dve_order.append(("recip", 0, h))
        dve_order.append(("mul_attn", 0, h))
    for h in range(GH):
        dve_order.append(("recip", 1, h))
        dve_order.append(("mul_attn", 1, h))
        dve_order.append(("copy_ut", h, 0))
        dve_order.append(("copy_ut", h, 1))
    for tq in range(TQ):
        for h in range(GH):
            dve_order.append(("mul_x", tq, h))
    for op in dve_order:
        vs_next(op)

    with nc.Block() as block:

        @block.sync
        def _(sync):
            for name in dma_in_order:
                dst = ident_sb if name == "identD" else ins_sb[name]
                nc.sync.dma_start(dst[:], dram_in[name][:]).then_inc(
                    dma_sem[name], 16
                )
            os_n = 0
            for tq in range(TQ):
                sync.wait_ge(VS, vs_at["mul_attn", tq, GH - 1])
                nc.sync.dma_start(
                    out_attn[tq * P : (tq + 1) * P, :, :], attn_sb[:, tq]
                ).then_inc(OS, 16)
                os_n += 16
            for tq in range(TQ):
                sync.wait_ge(VS, vs_at["mul_x", tq, GH - 1])
                nc.sync.dma_start(
                    out_x[tq * P : (tq + 1) * P, :], x_sb[:, tq]
                ).then_inc(OS, 16)
                os_n += 16
            sync.wait_ge(OS, os_n)

        @block.tensor
        def _(tensor):
            # HAM warmup (results overwritten later by start=True)
            tensor.wait_ge(*dma_gate["identD"])
            for _ in range(34):
                nc.tensor.matmul(
                    px[0][:, :P], lhsT=ident_sb[:, :], rhs=ident_sb[:, :],
                    start=True, stop=True,
                )
            proj_src = {
                "q": (ins_sb["wqT"], ins_sb["qT"], N),
                "k": (ins_sb["wkT"], ins_sb["kT"], N),
                "v": (ins_sb["vT"], ins_sb["wvT"], GC),
            }
            # VS value the S matmul of (tq=0, bank bi) must wait for:
            # data ready (qhT/khT copies) and psum bank free (pq copies)
            s_first_gate = {
                0: vs_at["copy_proj", "k", 0],
                1: vs_at["copy_proj", "k", 1],
                2: vs_at["copy_proj", "v", 0],
                3: vs_at["copy_proj", "v", 1],
            }
            s_banks = [ps[0], ps[1], pq[0], pq[1]]
            for op in pe_order:
                kind = op[0]
                if kind == "proj":
                    _, which, j = op
                    a_sb, b_sb, nfree = proj_src[which]
                    if j == 0:
                        tensor.wait_ge(*dma_gate[which])
                    prev_copy = {
                        ("q", 0): None, ("q", 1): None,
                        ("k", 0): ("copy_proj", "q", 0),
                        ("k", 1): ("copy_proj", "q", 1),
                        ("v", 0): ("copy_proj", "k", 0),
                        ("v", 1): ("copy_proj", "k", 1),
                    }[which, j]
                    if prev_copy is not None:
                        tensor.wait_ge(VS, vs_at[prev_copy])
                    for kc in range(KC):
                        i = nc.tensor.matmul(
                            pq[j][:, :nfree],
                            lhsT=a_sb[:, kc, j * P : (j + 1) * P],
                            rhs=b_sb[:, kc, :],
                            start=(kc == 0),
                            stop=(kc == KC - 1),
                        )
                    i.then_inc(PS, 1)
                elif kind == "S":
                    _, tq, h = op
                    bi = (tq * GH + h) % 4
                    cc = h // (P // HS)
                    r0 = (h % (P // HS)) * HS
                    if tq == 0:
                        tensor.wait_ge(VS, s_first_gate[bi])
                        if h >= 2:
                            tensor.wait_ge(VS, vs_at["copy_proj", "k", 1])
                    else:
                        tensor.wait_ge(AS, as_exp[0, h])
                    nc.tensor.matmul(
                        s_banks[bi][:, :N],
                        lhsT=qhT[r0 : r0 + HS, cc, tq * P : (tq + 1) * P],
                        rhs=khT[r0 : r0 + HS, cc, :],
                        start=True,
                        stop=True,
                    ).then_inc(PS, 1)
                elif kind == "T":
                    _, h, tk = op
                    if tk == 0:
                        tensor.wait_ge(AS, as_exp[1, h])
                        if h > 0:
                            tensor.wait_ge(VS, vs_at["copy_ut", h - 1, 1])
                    for tq in range(TQ):
                        i = nc.tensor.transpose(
                            pt[tk][:, tq * P : (tq + 1) * P],
                            u_sb[:, tq, h, tk * P : (tk + 1) * P],
                            ident_sb[:, :],
                        )
                    i.then_inc(PS, 1)
                elif kind == "AV":
                    _, tq, h = op
                    tensor.wait_ge(VS, vs_at["copy_ut", h, 1])
                    for tk in range(TK):
                        i = nc.tensor.matmul(
                            px[tq][:, h * HS : (h + 1) * HS],
                            lhsT=ut_sb[:, h, tk, tq * P : (tq + 1) * P],
                            rhs=vh[:, tk, h * HS : (h + 1) * HS],
                            start=(tk == 0),
                            stop=(tk == TK - 1),
                        )
                    i.then_inc(PS, 1)

        @block.scalar
        def _(scalar):
            # preload exp table during the DMA wait (operand: ident column)
            scalar.wait_ge(*dma_gate["identD"])
            nc.scalar.activation(scratch[:, 0:1], ident_sb[:, 0:1], Exp)
            s_banks = [ps[0], ps[1], pq[0], pq[1]]
            for tq in range(TQ):
                for h in range(GH):
                    bi = (tq * GH + h) % 4
                    idx = tq * GH + h
                    scalar.wait_ge(PS, ps_S[tq, h])
                    nc.scalar.activation(
                        u_sb[:, tq, h, :],
                        s_banks[bi][:, :N],
                        Exp,
                        scale=SCALE,
                        accum_out=ds_sb[:, idx : idx + 1],
                    ).then_inc(AS, 1)

        @block.vector
        def _(vector):
            proj_dst = {
                ("q", 0): qhT[:, 0, :], ("q", 1): qhT[:, 1, :],
                ("k", 0): khT[:, 0, :], ("k", 1): khT[:, 1, :],
                ("v", 0): vh[:, 0, :], ("v", 1): vh[:, 1, :],
            }
            for op in dve_order:
                kind = op[0]
                if kind == "copy_proj":
                    _, which, j = op
                    vector.wait_ge(PS, ps_proj[which, j])
                    nc.vector.tensor_copy(proj_dst[which, j], pq[j][:, : (
                        N if which != "v" else GC)]).then_inc(VS, 1)
                elif kind == "recip":
                    _, tq, h = op
                    idx = tq * GH + h
                    vector.wait_ge(AS, as_exp[tq, h])
                    nc.vector.reciprocal(
                        rc_sb[:, idx : idx + 1], ds_sb[:, idx : idx + 1]
                    ).then_inc(VS, 1)
                elif kind == "mul_attn":
                    _, tq, h = op
                    idx = tq * GH + h
                    # same-engine RAW on rc_sb: wait for the reciprocal's
                    # writeback (DVE pipeline overlaps otherwise)
                    vector.wait_ge(VS, vs_at["recip", tq, h])
                    nc.vector.tensor_scalar_mul(
                        attn_sb[:, tq, h, :], u_sb[:, tq, h, :],
                        rc_sb[:, idx : idx + 1],
                    ).then_inc(VS, 1)
                elif kind == "copy_ut":
                    _, h, tk = op
                    vector.wait_ge(PS, ps_T[h, tk])
                    nc.vector.tensor_copy(
                        ut_sb[:, h, tk, :], pt[tk][:]
                    ).then_inc(VS, 1)
                elif kind == "mul_x":
                    _, tq, h = op
                    idx = tq * GH + h
                    # wait for the LAST AV into this px bank: concurrent
                    # PE-write + DVE-read of one PSUM bank is a HW fault
                    vector.wait_ge(PS, ps_AV[tq, GH - 1])
                    vector.wait_ge(VS, vs_at["recip", tq, h])
                    nc.vector.tensor_scalar_mul(
                        x_sb[:, tq, h * HS : (h + 1) * HS],
                        px[tq][:, h * HS : (h + 1) * HS],
                        rc_sb[:, idx : idx + 1],
                    ).then_inc(VS, 1)

    nc.compile()
    return nc


def _get_nc(dt_mm_name: str):
    key = (MODE, dt_mm_name)
    if key not in _CACHE:
        _CACHE[key] = (
            _build_nc_raw(dt_mm_name) if MODE == "raw" else _build_nc(dt_mm_name)
        )
    return _CACHE[key]


def _pack(a, dtype):
    # [512, F] -> [128, 4, F] partition-major contiguous
    F = a.shape[1]
    return np.ascontiguousarray(
        a.reshape(KC, P, F).transpose(1, 0, 2).astype(dtype)
    )


def make_in_maps(q, k, v, W_q, W_k, W_v, dt_mm_name=None):
    """Shard full inputs into 8 per-core input dicts (host-side transposes)."""
    dt = _np_in_dtype(dt_mm_name or DT_MM)
    in_maps = []
    packed_w = {}
    for hg in range(HG):
        sl = slice(hg * GC, (hg + 1) * GC)
        packed_w[hg] = (
            _pack(np.ascontiguousarray(W_q[sl, :].T), dt),
            _pack(np.ascontiguousarray(W_k[sl, :].T), dt),
            _pack(np.ascontiguousarray(W_v[sl, :].T), dt),
        )
    packed_x = {}
    for b in range(B):
        packed_x[b] = (
            _pack(np.ascontiguousarray(q[b].T), dt),
            _pack(np.ascontiguousarray(k[b].T), dt),
            _pack(np.ascontiguousarray(v[b].T), dt),
        )
    ident = np.eye(P, dtype=dt) if MODE == "raw" else None
    for c in range(2 * B):
        b, hg = c // HG, c % HG
        qTp, kTp, vTp = packed_x[b]
        wqTp, wkTp, wvTp = packed_w[hg]
        m = {"qT": qTp, "kT": kTp, "vT": vTp, "wqT": wqTp, "wkT": wkTp, "wvT": wvTp}
        if ident is not None:
            m["identD"] = ident
        in_maps.append(m)
    return in_maps


def assemble(results):
    """Gather 8 per-core outputs into full (x, attn)."""
    x = np.empty((B, N, C), dtype=np.float32)
    attn = np.empty((B, H, N, N), dtype=np.float32)
    for c in range(2 * B):
        b, hg = c // HG, c % HG
        # out_attn [N, GH, N] -> [GH, N, N]
        attn[b, hg * GH : (hg + 1) * GH] = np.asarray(
            results[c]["out_attn"], dtype=np.float32
        ).transpose(1, 0, 2)
        x[b, :, hg * GC : (hg + 1) * GC] = results[c]["out_x"]
    return x, attn


def kernel(q, k, v, relation_feature=None, W_q=None, W_k=None, W_v=None,
           W_r_conv=None, W_r_qk=None, _trace=False):
    from concourse.bass_utils import run_bass_kernel_spmd

    nc = _get_nc(DT_MM)
    in_maps = make_in_maps(
        np.asarray(q), np.asarray(k), np.asarray(v),
        np.asarray(W_q), np.asarray(W_k), np.asarray(W_v),
    )
    res = run_bass_kernel_spmd(nc, in_maps, core_ids=list(range(2 * B)), trace=_trace)
    x, attn = assemble(res.results)
    if _trace:
        return (x, attn), res
    return (x, attn)


# revision 23
# speedup vs baseline: 1.0381x; 1.0381x over previous
"""Multi-head attention (sparse_attention nn_Attention) on 8 TRN2 NeuronCores.

Reference computes standard MHA: project q/k/v, S = qh @ kh^T, attn =
softmax(S * scale), x = attn @ vh; returns (x, attn). The relation_feature
branch in the reference is dead code (computed then deleted), so
relation_feature is never touched here.

Sharding: 8 cores = 4 batches x 2 head-groups. Core c handles batch c//2,
heads [4*(c%2), 4*(c%2)+4). Weight slices per head-group, activations per
batch. No collectives: every core writes a disjoint output slice.

Host pre-transposes inputs so every matmul contraction lands on SBUF
partitions, packed [128, KC, 256] partition-major so each DMA partition line
is one contiguous chunk:
  qT/kT/vT  <- q[b].T              (C=512 rows -> KC=4 chunks of 128)
  w{q,k,v}T <- W[hg*256:+256, :].T
Per-core compute (querytok on partitions through softmax):
  qhT/khT [chan, tok], vh [tok, chan] projections on PE
  S[h]    [qtok, ktok] = qhT_h.T @ khT_h      (K=64 per head)
  U       = exp(S*scale) on ACT with fused row-sum (accum_out)
  attn    = U * (1/rowsum)  -> packed [qtok, h, ktok] f32 out
  UT      = PE-transpose(U);  xu = UT.T @ vh;  x = xu * (1/rowsum)
Matmul dtype modes: "float32" (4 cyc/row), "bfloat16" (1 cyc/row, bf16 DMA),
"float32r" (f32 storage bitcast to the fast PE path, 1 cyc/row at N>=256).
"""

import numpy as np

B, N, C = 4, 256, 512
H, HS = 8, 64
HG = 2  # head groups (tensor-parallel over heads)
GH = H // HG  # heads per group = 4
GC = GH * HS  # channels per group = 256
SCALE = HS**-0.5
P = 128
KC = C // P  # 4 contraction chunks
TQ = N // P  # 2 query-token chunks
TK = N // P  # 2 key-token chunks
CCH = GC // P  # 2 channel chunks per group

DT_MM = "bfloat16"  # "float32" | "bfloat16" | "float32r"
MODE = "raw"  # "raw" (manual semaphores) | "tile" (TileContext)

_CACHE = {}


def _np_in_dtype(dt_mm_name):
    if dt_mm_name == "bfloat16":
        import ml_dtypes

        return ml_dtypes.bfloat16
    return np.float32


def _make_slim_tile(tile):
    """TileContext whose exit skips the two all-engine barriers and ~57
    semaphore clears (~8us tail). Keeps the sync-engine drain (with waits on
    the global clock) so output DMAs complete before the NEFF ends. The NEFF
    then assumes clean semaphores at load (single execution per load, which
    is how run_bass_via_pjrt executes it)."""
    from concourse.vector_clock import ScopedClock

    class SlimTile(tile.TileContext):
        def _drain_and_barrier(self, tick_clock, wait_clock):
            drain_inst = self.nc.sync.drain()
            wait_clock.add_sem_waits(
                drain_inst.ins, ScopedClock({None: tick_clock.global_clock})
            )
            popped = self.nc._tile_sem_poison_stack.pop()
            assert popped is self._sem_poison

    return SlimTile


def _build_nc(dt_mm_name: str):
    import concourse.bass as bass  # noqa: F401
    import concourse.mybir as mybir
    import concourse.tile as tile
    from concourse import bacc
    from concourse.masks import make_identity

    f32 = mybir.dt.float32
    dt_mm = getattr(mybir.dt, dt_mm_name)
    # dtype of DRAM inputs + SBUF input tiles; f32r is stored as f32 in
    # SBUF/PSUM and bitcast to float32r at each matmul AP
    dt_in = mybir.dt.bfloat16 if dt_mm_name == "bfloat16" else f32
    dt_store = mybir.dt.bfloat16 if dt_mm_name == "bfloat16" else f32

    def mm_cast(ap):
        return ap.bitcast(dt_mm) if dt_mm_name == "float32r" else ap

    nc = bacc.Bacc("TRN2", target_bir_lowering=False)

    qT = nc.dram_tensor("qT", [P, KC, N], dt_in, kind="ExternalInput")
    kT = nc.dram_tensor("kT", [P, KC, N], dt_in, kind="ExternalInput")
    vT = nc.dram_tensor("vT", [P, KC, N], dt_in, kind="ExternalInput")
    wqT = nc.dram_tensor("wqT", [P, KC, GC], dt_in, kind="ExternalInput")
    wkT = nc.dram_tensor("wkT", [P, KC, GC], dt_in, kind="ExternalInput")
    wvT = nc.dram_tensor("wvT", [P, KC, GC], dt_in, kind="ExternalInput")
    # attn packed [querytok, head, keytok] so DMA rows are 4KB; host unpacks
    out_attn = nc.dram_tensor("out_attn", [N, GH, N], f32, kind="ExternalOutput")
    out_x = nc.dram_tensor("out_x", [N, GC], f32, kind="ExternalOutput")

    SlimTile = _make_slim_tile(tile)
    with SlimTile(nc) as tc:
        with (
            tc.tile_pool(name="inputs", bufs=1) as inp,
            tc.tile_pool(name="proj", bufs=1) as proj,
            tc.tile_pool(name="work", bufs=2) as work,
            tc.tile_pool(name="small", bufs=16) as small,
            tc.tile_pool(name="psA", bufs=2, space="PSUM") as psA,
            tc.tile_pool(name="psB", bufs=2, space="PSUM") as psB,
            tc.tile_pool(name="psC", bufs=2, space="PSUM") as psC,
        ):
            ident = inp.tile([P, P], dt_store)
            make_identity(nc, ident)

            def load(t, fdim):
                sb = inp.tile([P, KC, fdim], dt_in, tag=f"in_{t.name}")
                nc.sync.dma_start(sb[:], t[:])
                return sb

            wqT_sb = load(wqT, GC)
            qT_sb = load(qT, N)
            wkT_sb = load(wkT, GC)
            kT_sb = load(kT, N)
            wvT_sb = load(wvT, GC)
            vT_sb = load(vT, N)

            # ---- projections
            qhT = proj.tile([P, CCH, N], dt_store)  # [chan_part, cc, querytok]
            khT = proj.tile([P, CCH, N], dt_store)
            vh = proj.tile([P, TK, GC], dt_store)  # [keytok_part, tk, chan]

            def project(dst_slice, w_sb, x_sb, m_sl, nfree):
                ps_full = psA.tile([P, 512], f32, tag="proj_ps", name="proj_ps")
                ps = ps_full[:, :nfree]
                for kc in range(KC):
                    nc.tensor.matmul(
                        ps,
                        lhsT=mm_cast(w_sb[:, kc, m_sl]),
                        rhs=mm_cast(x_sb[:, kc, :]),
                        start=(kc == 0),
                        stop=(kc == KC - 1),
                    )
                nc.any.tensor_copy(dst_slice, ps)

            for cc in range(CCH):
                sl = slice(cc * P, (cc + 1) * P)
                project(qhT[:, cc, :], wqT_sb, qT_sb, sl, N)
                project(khT[:, cc, :], wkT_sb, kT_sb, sl, N)
            for tk in range(TK):
                sl = slice(tk * P, (tk + 1) * P)
                project(vh[:, tk, :], vT_sb, wvT_sb, sl, GC)

            # ---- attention: S, exp(+rowsum), normalize, write attn
            u_all = proj.tile([P, TQ, GH, N], dt_store)  # exp(S*scale)
            ut_all = proj.tile([P, GH, TK, TQ * P], dt_store)  # U^T
            recips = {}
            for tq in range(TQ):
                attn_f = work.tile([P, GH, N], f32, tag="attn_f")
                for h in range(GH):
                    cc = h // (P // HS)
                    r0 = (h % (P // HS)) * HS
                    s_ps = psA.tile([P, N], f32, tag="s_ps")
                    nc.tensor.matmul(
                        s_ps,
                        lhsT=mm_cast(qhT[r0 : r0 + HS, cc, tq * P : (tq + 1) * P]),
                        rhs=mm_cast(khT[r0 : r0 + HS, cc, :]),
                        start=True,
                        stop=True,
                    )
                    u = u_all[:, tq, h, :]
                    dsum = small.tile([P, 1], f32, tag="dsum")
                    nc.scalar.activation(
                        u,
                        s_ps,
                        mybir.ActivationFunctionType.Exp,
                        scale=SCALE,
                        accum_out=dsum,
                    )
                    rc = small.tile([P, 1], f32, tag="recip", name=f"rc_{tq}_{h}")
                    nc.vector.reciprocal(rc, dsum)
                    recips[tq, h] = rc
                    nc.vector.tensor_scalar_mul(attn_f[:, h, :], u, rc)
                nc.sync.dma_start(out_attn[tq * P : (tq + 1) * P, :, :], attn_f[:])

            # ---- transpose unnormalized U: [qtok, ktok] -> [ktok, qtok]
            for h in range(GH):
                for tk in range(TK):
                    t_ps = psB.tile([P, TQ * P], dt_store, tag="t_ps")
                    for tq in range(TQ):
                        nc.tensor.transpose(
                            mm_cast(t_ps[:, tq * P : (tq + 1) * P]),
                            mm_cast(u_all[:, tq, h, tk * P : (tk + 1) * P]),
                            mm_cast(ident[:, :]),
                        )
                    nc.any.tensor_copy(ut_all[:, h, tk, :], t_ps)

            # ---- x = attn @ vh (unnormalized, then scaled by recip)
            for tq in range(TQ):
                x_ps = psC.tile([P, GC], f32, tag="x_ps")
                for h in range(GH):
                    for tk in range(TK):
                        nc.tensor.matmul(
                            x_ps[:, h * HS : (h + 1) * HS],
                            lhsT=mm_cast(ut_all[:, h, tk, tq * P : (tq + 1) * P]),
                            rhs=mm_cast(vh[:, tk, h * HS : (h + 1) * HS]),
                            start=(tk == 0),
                            stop=(tk == TK - 1),
                        )
                x_sb = work.tile([P, GC], f32, tag="x_sb")
                for h in range(GH):
                    nc.vector.tensor_scalar_mul(
                        x_sb[:, h * HS : (h + 1) * HS],
                        x_ps[:, h * HS : (h + 1) * HS],
                        recips[tq, h],
                    )
                nc.sync.dma_start(out_x[tq * P : (tq + 1) * P, :], x_sb[:])

    nc.compile()
    return nc


def _build_nc_raw(dt_mm_name: str):
    """Raw bacc build: one Block, five engine streams, manual semaphores.

    vs the Tile build: no per-instruction semaphore plumbing, no Block-end
    all-engine barrier (the sync stream's final wait on the output-DMA
    semaphore already proves all bytes landed), PE pre-warmed via dummy
    matmuls on a gpsimd-memset tile (HAM ramps to 2.4 GHz during the DMA
    wait), identity built on-chip, inputs packed in 3 pair-DMAs issued from
    three different DGE rings (sync/scalar/gpsimd) to dodge the ~600ns
    per-dma_start descriptor-generation serialization.
    """
    assert dt_mm_name == "bfloat16", "raw mode is bf16-only"
    import concourse.mybir as mybir
    from concourse.bass import BassBlock

    from concourse import bacc

    f32 = mybir.dt.float32
    bf16 = mybir.dt.bfloat16
    Exp = mybir.ActivationFunctionType.Exp

    nc = bacc.Bacc("TRN2", target_bir_lowering=False)

    in1 = nc.dram_tensor("in1", [P, 2, KC, N], bf16, kind="ExternalInput")  # wq,q
    in2 = nc.dram_tensor("in2", [P, 2, KC, N], bf16, kind="ExternalInput")  # wk,k
    in3 = nc.dram_tensor("in3", [P, 2, KC, N], bf16, kind="ExternalInput")  # wv,v
    out_attn = nc.dram_tensor("out_attn", [N, GH, N], f32, kind="ExternalOutput")
    out_x = nc.dram_tensor("out_x", [N, GC], f32, kind="ExternalOutput")

    # SBUF
    in1_sb = nc.alloc_sbuf_tensor("in1_sb", [P, 2, KC, N], bf16)
    in2_sb = nc.alloc_sbuf_tensor("in2_sb", [P, 2, KC, N], bf16)
    in3_sb = nc.alloc_sbuf_tensor("in3_sb", [P, 2, KC, N], bf16)
    ident_sb = nc.alloc_sbuf_tensor("ident_sb", [P, P], bf16)
    warm_sb = nc.alloc_sbuf_tensor("warm_sb", [P, P], bf16)
    qhT = nc.alloc_sbuf_tensor("qhT", [P, CCH, N], bf16)
    khT = nc.alloc_sbuf_tensor("khT", [P, CCH, N], bf16)
    vh = nc.alloc_sbuf_tensor("vh", [P, TK, GC], bf16)
    u_sb = nc.alloc_sbuf_tensor("u_sb", [P, TQ, GH, N], bf16)
    ut_sb = nc.alloc_sbuf_tensor("ut_sb", [P, GH, TK, TQ * P], bf16)
    attn_sb = nc.alloc_sbuf_tensor("attn_sb", [P, TQ, GH, N], f32)
    x_sb = nc.alloc_sbuf_tensor("x_sb", [P, TQ, GC], f32)
    ds_sb = nc.alloc_sbuf_tensor("ds_sb", [P, TQ * GH], f32)
    rc_sb = nc.alloc_sbuf_tensor("rc_sb", [P, TQ * GH], f32)
    scratch = nc.alloc_sbuf_tensor("scratch", [P, 1], f32)

    # PSUM: exactly 8 banks
    pq = [nc.alloc_psum_tensor(f"pq{i}", [P, 512], f32) for i in range(2)]
    ps = [nc.alloc_psum_tensor(f"ps{i}", [P, N], f32) for i in range(2)]
    pt = [nc.alloc_psum_tensor(f"pt{i}", [P, TQ * P], bf16) for i in range(2)]
    px = [nc.alloc_psum_tensor(f"px{i}", [P, GC], f32) for i in range(2)]

    DQ = nc.alloc_semaphore("DQ")  # in1 (wq,q)
    DK = nc.alloc_semaphore("DK")  # in2 (wk,k)
    DV = nc.alloc_semaphore("DV")  # in3 (wv,v)
    GS = nc.alloc_semaphore("GS")  # gpsimd milestones (warm, ident)
    OS = nc.alloc_semaphore("OS")  # output DMAs (HWDGE rings)
    OX = nc.alloc_semaphore("OX")  # x output DMA (gpsimd SWDGE needs own sem)
    PS = nc.alloc_semaphore("PS")  # tensor engine milestones
    AS = nc.alloc_semaphore("AS")  # scalar exps
    VS = nc.alloc_semaphore("VS")  # vector ops

    w_pair = {"q": in1_sb, "k": in2_sb, "v": in3_sb}
    dma_gate = {"q": (DQ, 16), "k": (DK, 16), "v": (DV, 16)}

    # PE program order; PS semaphore value = 1 + index in this list
    pe_order = []
    for which in ("q", "k", "v"):
        for j in range(2):
            pe_order.append(("proj", which, j))
    for tq in range(TQ):
        for h in range(GH):
            pe_order.append(("S", tq, h))
    for h in range(GH):
        pe_order.append(("T", h, 0))
        pe_order.append(("T", h, 1))
        if h > 0:
            pe_order.append(("AV", 0, h - 1))
    pe_order.append(("AV", 0, GH - 1))
    for h in range(GH):
        pe_order.append(("AV", 1, h))
    ps_at = {op: i + 1 for i, op in enumerate(pe_order)}
    ps_proj = {(w, j): ps_at["proj", w, j] for w in ("q", "k", "v") for j in range(2)}
    ps_S = {(tq, h): ps_at["S", tq, h] for tq in range(TQ) for h in range(GH)}
    ps_T = {(h, tk): ps_at["T", h, tk] for h in range(GH) for tk in range(TK)}
    ps_AV = {(tq, h): ps_at["AV", tq, h] for tq in range(TQ) for h in range(GH)}

    as_exp = {}
    n = 0
    for tq in range(TQ):
        for h in range(GH):
            n += 1
            as_exp[tq, h] = n

    # DVE program order; VS value = 1 + index
    dve_order = []
    for which, j in (("q", 0), ("q", 1), ("k", 0), ("k", 1), ("v", 0), ("v", 1)):
        dve_order.append(("copy_proj", which, j))
    for h in range(GH):
        dve_order.append(("recip", 0, h))
        dve_order.append(("mul_attn", 0, h))
    for h in range(GH):
        dve_order.append(("recip", 1, h))
        dve_order.append(("mul_attn", 1, h))
        dve_order.append(("copy_ut", h, 0))
        dve_order.append(("copy_ut", h, 1))
    for tq in range(TQ):
        for h in range(GH):
            dve_order.append(("mul_x", tq, h))
    vs_at = {op: i + 1 for i, op in enumerate(dve_order)}

    blk = BassBlock(nc, "blk")
    nc.cur_block = blk
    blk.__enter__()

    def sync_fn(sync):
        nc.sync.dma_start(in1_sb[:], in1[:]).then_inc(DQ, 16)
        sync.wait_ge(VS, vs_at["mul_attn", 0, GH - 1])
        nc.sync.dma_start(out_attn[0:P, :, :], attn_sb[:, 0]).then_inc(OS, 16)
        sync.wait_ge(OS, 32)  # attn0 + attn1 landed
        sync.wait_ge(OX, 16)  # x landed

    def gpsimd_fn(gpsimd):
        nc.gpsimd.memset(warm_sb[:, :], 0.0).then_inc(GS, 1)
        nc.gpsimd.dma_start(in3_sb[:], in3[:]).then_inc(DV, 16)
        nc.gpsimd.memset(ident_sb[:, :], 0.0)
        nc.gpsimd.drain()
        nc.gpsimd.affine_select(
            out=ident_sb[:, :],
            in_=ident_sb[:, :],
            compare_op=mybir.AluOpType.not_equal,
            fill=1.0,
            base=0,
            pattern=[[-1, P]],
            channel_multiplier=1,
        ).then_inc(GS, 1)
        gpsimd.wait_ge(VS, vs_at["mul_x", 1, GH - 1])
        nc.gpsimd.dma_start(
            out_x.rearrange("(t p) c -> p t c", p=P), x_sb[:, :, :]
        ).then_inc(OX, 16)

    def tensor_fn(tensor):
        tensor.wait_ge(GS, 1)
        for _ in range(30):
            nc.tensor.matmul(
                px[0][:, :P], lhsT=warm_sb[:, :], rhs=warm_sb[:, :],
                start=True, stop=True,
            )
        s_first_gate = {
            0: vs_at["copy_proj", "k", 0],
            1: vs_at["copy_proj", "k", 1],
            2: vs_at["copy_proj", "v", 0],
            3: vs_at["copy_proj", "v", 1],
        }
        s_banks = [ps[0], ps[1], pq[0], pq[1]]
        for op in pe_order:
            kind = op[0]
            if kind == "proj":
                _, which, j = op
                pair = w_pair[which]
                if j == 0:
                    tensor.wait_ge(*dma_gate[which])
                prev_copy = {
                    ("q", 0): None, ("q", 1): None,
                    ("k", 0): ("copy_proj", "q", 0),
                    ("k", 1): ("copy_proj", "q", 1),
                    ("v", 0): ("copy_proj", "k", 0),
                    ("v", 1): ("copy_proj", "k", 1),
                }[which, j]
                if prev_copy is not None:
                    tensor.wait_ge(VS, vs_at[prev_copy])
                # pair[:, 0] = weightT, pair[:, 1] = activationT; for v the
                # lhsT is the activation (vT) and the rhs is the weight
                if which == "v":
                    a_sel, b_sel = 1, 0
                else:
                    a_sel, b_sel = 0, 1
                for kc in range(KC):
                    i = nc.tensor.matmul(
                        pq[j][:, :N],
                        lhsT=pair[:, a_sel, kc, j * P : (j + 1) * P],
                        rhs=pair[:, b_sel, kc, :],
                        start=(kc == 0),
                        stop=(kc == KC - 1),
                    )
                i.then_inc(PS, 1)
            elif kind == "S":
                _, tq, h = op
                bi = (tq * GH + h) % 4
                cc = h // (P // HS)
                r0 = (h % (P // HS)) * HS
                if tq == 0:
                    tensor.wait_ge(VS, s_first_gate[bi])
                    if h >= 2:
                        tensor.wait_ge(VS, vs_at["copy_proj", "k", 1])
                else:
                    tensor.wait_ge(AS, as_exp[0, h])
                nc.tensor.matmul(
                    s_banks[bi][:, :N],
                    lhsT=qhT[r0 : r0 + HS, cc, tq * P : (tq + 1) * P],
                    rhs=khT[r0 : r0 + HS, cc, :],
                    start=True,
                    stop=True,
                ).then_inc(PS, 1)
            elif kind == "T":
                _, h, tk = op
                if tk == 0:
                    tensor.wait_ge(AS, as_exp[1, h])
                    if h == 0:
                        tensor.wait_ge(GS, 2)  # identity built
                    else:
                        tensor.wait_ge(VS, vs_at["copy_ut", h - 1, 1])
                for tq in range(TQ):
                    i = nc.tensor.transpose(
                        pt[tk][:, tq * P : (tq + 1) * P],
                        u_sb[:, tq, h, tk * P : (tk + 1) * P],
                        ident_sb[:, :],
                    )
                i.then_inc(PS, 1)
            elif kind == "AV":
                _, tq, h = op
                tensor.wait_ge(VS, vs_at["copy_ut", h, 1])
                for tk in range(TK):
                    i = nc.tensor.matmul(
                        px[tq][:, h * HS : (h + 1) * HS],
                        lhsT=ut_sb[:, h, tk, tq * P : (tq + 1) * P],
                        rhs=vh[:, tk, h * HS : (h + 1) * HS],
                        start=(tk == 0),
                        stop=(tk == TK - 1),
                    )
                i.then_inc(PS, 1)

    def scalar_fn(scalar):
        nc.scalar.dma_start(in2_sb[:], in2[:]).then_inc(DK, 16)
        # preload exp table during the DMA wait
        scalar.wait_ge(GS, 1)
        nc.scalar.activation(scratch[:, 0:1], warm_sb[:, 0:1], Exp)
        s_banks = [ps[0], ps[1], pq[0], pq[1]]
        for tq in range(TQ):
            for h in range(GH):
                bi = (tq * GH + h) % 4
                idx = tq * GH + h
                scalar.wait_ge(PS, ps_S[tq, h])
                nc.scalar.activation(
                    u_sb[:, tq, h, :],
                    s_banks[bi][:, :N],
                    Exp,
                    scale=SCALE,
                    accum_out=ds_sb[:, idx : idx + 1],
                ).then_inc(AS, 1)
        scalar.wait_ge(VS, vs_at["mul_attn", 1, GH - 1])
        nc.scalar.dma_start(out_attn[P : 2 * P, :, :], attn_sb[:, 1]).then_inc(
            OS, 16
        )

    def vector_fn(vector):
        proj_dst = {
            ("q", 0): qhT[:, 0, :], ("q", 1): qhT[:, 1, :],
            ("k", 0): khT[:, 0, :], ("k", 1): khT[:, 1, :],
            ("v", 0): vh[:, 0, :], ("v", 1): vh[:, 1, :],
        }
        for op in dve_order:
            kind = op[0]
            if kind == "copy_proj":
                _, which, j = op
                vector.wait_ge(PS, ps_proj[which, j])
                nc.vector.tensor_copy(proj_dst[which, j], pq[j][:, :N]).then_inc(
                    VS, 1
                )
            elif kind == "recip":
                _, tq, h = op
                idx = tq * GH + h
                vector.wait_ge(AS, as_exp[tq, h])
                nc.vector.reciprocal(
                    rc_sb[:, idx : idx + 1], ds_sb[:, idx : idx + 1]
                ).then_inc(VS, 1)
            elif kind == "mul_attn":
                _, tq, h = op
                idx = tq * GH + h
                # same-engine RAW on rc_sb: wait for the reciprocal writeback
                vector.wait_ge(VS, vs_at["recip", tq, h])
                nc.vector.tensor_scalar_mul(
                    attn_sb[:, tq, h, :], u_sb[:, tq, h, :],
                    rc_sb[:, idx : idx + 1],
                ).then_inc(VS, 1)
            elif kind == "copy_ut":
                _, h, tk = op
                vector.wait_ge(PS, ps_T[h, tk])
                nc.vector.tensor_copy(ut_sb[:, h, tk, :], pt[tk][:]).then_inc(
                    VS, 1
                )
            elif kind == "mul_x":
                _, tq, h = op
                idx = tq * GH + h
                # wait for the LAST AV into this px bank: concurrent
                # PE-write + DVE-read of one PSUM bank is a HW fault
                vector.wait_ge(PS, ps_AV[tq, GH - 1])
                vector.wait_ge(VS, vs_at["recip", tq, h])
                nc.vector.tensor_scalar_mul(
                    x_sb[:, tq, h * HS : (h + 1) * HS],
                    px[tq][:, h * HS : (h + 1) * HS],
                    rc_sb[:, idx : idx + 1],
                ).then_inc(VS, 1)

    blk.sync(sync_fn)
    blk.gpsimd(gpsimd_fn)
    blk.tensor(tensor_fn)
    blk.scalar(scalar_fn)
    blk.vector(vector_fn)

    # manual block exit WITHOUT the all-engine barrier (the OS wait on the
    # sync stream already guarantees output completion)
    for engine, last_body in blk.last_body.items():
        with nc.body(last_body, parent=nc.cur_bb, allow_existing_parent=True):
            engine.br(blk.end_bb)
    nc.switch_bb(blk.end_bb)
    nc.cur_block = None

    nc.compile()
    return nc


def _get_nc(dt_mm_name: str):
    key = (MODE, dt_mm_name)
    if key not in _CACHE:
        _CACHE[key] = (
            _build_nc_raw(dt_mm_name) if MODE == "raw" else _build_nc(dt_mm_name)
        )
    return _CACHE[key]


def _pack(a, dtype):
    # [512, F] -> [128, 4, F] partition-major contiguous
    F = a.shape[1]
    return np.ascontiguousarray(
        a.reshape(KC, P, F).transpose(1, 0, 2).astype(dtype)
    )


def make_in_maps(q, k, v, W_q, W_k, W_v, dt_mm_name=None):
    """Shard full inputs into 8 per-core input dicts (host-side transposes)."""
    dt = _np_in_dtype(dt_mm_name or DT_MM)
    in_maps = []
    packed_w = {}
    for hg in range(HG):
        sl = slice(hg * GC, (hg + 1) * GC)
        packed_w[hg] = (
            _pack(np.ascontiguousarray(W_q[sl, :].T), dt),
            _pack(np.ascontiguousarray(W_k[sl, :].T), dt),
            _pack(np.ascontiguousarray(W_v[sl, :].T), dt),
        )
    packed_x = {}
    for b in range(B):
        packed_x[b] = (
            _pack(np.ascontiguousarray(q[b].T), dt),
            _pack(np.ascontiguousarray(k[b].T), dt),
            _pack(np.ascontiguousarray(v[b].T), dt),
        )
    for c in range(2 * B):
        b, hg = c // HG, c % HG
        qTp, kTp, vTp = packed_x[b]
        wqTp, wkTp, wvTp = packed_w[hg]
        if MODE == "raw":
            in_maps.append(
                {
                    "in1": np.ascontiguousarray(np.stack([wqTp, qTp], axis=1)),
                    "in2": np.ascontiguousarray(np.stack([wkTp, kTp], axis=1)),
                    "in3": np.ascontiguousarray(np.stack([wvTp, vTp], axis=1)),
                }
            )
        else:
            in_maps.append(
                {"qT": qTp, "kT": kTp, "vT": vTp,
                 "wqT": wqTp, "wkT": wkTp, "wvT": wvTp}
            )
    return in_maps


def assemble(results):
    """Gather 8 per-core outputs into full (x, attn)."""
    x = np.empty((B, N, C), dtype=np.float32)
    attn = np.empty((B, H, N, N), dtype=np.float32)
    for c in range(2 * B):
        b, hg = c // HG, c % HG
        # out_attn [N, GH, N] -> [GH, N, N]
        attn[b, hg * GH : (hg + 1) * GH] = np.asarray(
            results[c]["out_attn"], dtype=np.float32
        ).transpose(1, 0, 2)
        x[b, :, hg * GC : (hg + 1) * GC] = results[c]["out_x"]
    return x, attn


def kernel(q, k, v, relation_feature=None, W_q=None, W_k=None, W_v=None,
           W_r_conv=None, W_r_qk=None, _trace=False):
    from concourse.bass_utils import run_bass_kernel_spmd

    nc = _get_nc(DT_MM)
    in_maps = make_in_maps(
        np.asarray(q), np.asarray(k), np.asarray(v),
        np.asarray(W_q), np.asarray(W_k), np.asarray(W_v),
    )
    res = run_bass_kernel_spmd(nc, in_maps, core_ids=list(range(2 * B)), trace=_trace)
    x, attn = assemble(res.results)
    if _trace:
        return (x, attn), res
    return (x, attn)
